# revision 5
# baseline (speedup 1.0000x reference)
"""Trainium2 Bass kernel for nn_Attention_28020366639391 (sparse attention), v2.

Math (per batch element b, reference semantics):
    q/k/v = x @ W{q,k,v} + b, 12 heads of 64; scores = q k^T / 8
    rows >= 512 zeroed pre-softmax -> those ctx rows = mean_k(v)
    out = concat_heads(ctx) @ Wo + bo

Sharding: data-parallel on batch, 8 elements -> 8 cores, no collectives.

Per-core dataflow (v2):
  Host prep: xT in fp8 hi/lo ([128, 6, 1024], partition = d_in%128), weights
  pre-scaled by 64 then split fp8 hi/lo ([128, 6, 768]); Wo in fp16.
  - QKV projections: fp8 DoubleRow, 3 products (hh + hl + lh) ~ bf16-accurate
    at 0.5 cycles/row. QT/KT evac to fp16 [d_out, q]; V evac to fp16
    Vaug [keys, head*65] with a 64.0 column per head (softmax denominator,
    absorbing the x64 weight scale).
  - scores (per head, per 128-key chunk): full-rate fp16 matmul
    KT_h^T-slice @ QT_h -> PSUM [keys, 512]; exp via ACT (scale 2^-15)
    or Schraudolph int16->fp16 bit trick on DVE/Pool; output fp16 e.
  - ctx (layout B, kc-major): out[q,65] += e_slice^T @ Vaug_h per key chunk
    (65-wide moving operand: 8x fewer PE cycles than the [*,q] layout).
  - normalize fused into PSUM evac: ctx * recip(denom) -> sbctx [q, d] fp16.
  - PE-transpose sbctx -> ctxT [d, q] fp16; out-proj ctxT^T-slices @ Wo.
  - tail rows 512:1024 = broadcast of (mean_k v) @ Wo; mean_k v comes from a
    ones^T @ Vaug PE reduction (uses the accurate hi/lo V, no DVE reduce).
  DMAs are spread across the SP/ACT/DVE/Pool/PE DGE queues so input loading
  and output drain run as parallel streams.
"""

import numpy as np

import concourse.bass as bass
import concourse.mybir as mybir
import concourse.tile as tile
from concourse import bacc
from concourse.bass_utils import run_bass_kernel_spmd
from concourse.masks import make_identity

B, S, D, H, DH = 8, 1024, 768, 12, 64
SH = 512            # active query rows
DC = D // 128       # 6 chunks of model dim
SC = S // 128       # 8 chunks of sequence dim
NCORES = 8
WS = 64.0           # host-side weight scaling (fp8 lo-residual range fix)
FP = mybir.dt.float32
F16 = mybir.dt.float16
F8 = mybir.dt.float8e4
I16 = mybir.dt.int16
AF = mybir.ActivationFunctionType
ALU = mybir.AluOpType
DR = mybir.MatmulPerfMode.DoubleRow

LOG2E = 1.4426950408889634
SCORE_SCALE = 0.125 / (WS * WS)      # folded 1/64^2 weight scaling
SCH_A = SCORE_SCALE * LOG2E * 1024.0 # schraudolph int16 multiplier
SCH_B = 15.0 * 1024.0 - 44.0         # schraudolph bias (tuned C=44, floor conv)

# exp engine per (h*8+kc) % 8: a=ACT(exact), d=DVE, p=Pool (schraudolph)
EXP_SCHED = "adpdadpa"
# normalize/evac engine per head (ACT/Pool; DVE handles recips + mvt)
NRM_SCHED = "ap"
# V-evac engine per sequence chunk
VEV_SCHED = "adpadpad"


def _mm_hilo(nc, out, lhs_hi, lhs_lo, rhs_hi, rhs_lo, first, last,
             lo_last="lhs"):
    """3-product hi/lo fp8 DoubleRow accumulation into one PSUM group.

    lo_last picks which lo-product goes last, so the operand that arrives
    last on the serial DMA stream doesn't stall the group start."""
    nc.tensor.matmul(out, lhs_hi, rhs_hi, start=first, stop=False, perf_mode=DR)
    if lo_last == "lhs":
        nc.tensor.matmul(out, lhs_hi, rhs_lo, start=False, stop=False,
                         perf_mode=DR)
        nc.tensor.matmul(out, lhs_lo, rhs_hi, start=False, stop=last,
                         perf_mode=DR)
    else:
        nc.tensor.matmul(out, lhs_lo, rhs_hi, start=False, stop=False,
                         perf_mode=DR)
        nc.tensor.matmul(out, lhs_hi, rhs_lo, start=False, stop=last,
                         perf_mode=DR)


PHASES = []


def _body(tc, out, t_in, with_bias=False):
    nc = tc.nc
    from contextlib import ExitStack

    def mark(nm):
        PHASES.append((nm, len(list(nc.all_instructions()))))

    with ExitStack() as ctx:
        ctx.enter_context(
            nc.allow_low_precision(reason="fp8 hi/lo + fp16 pipeline by design")
        )
        constp = ctx.enter_context(tc.tile_pool(name="const", bufs=1))
        wp = ctx.enter_context(tc.tile_pool(name="wp", bufs=1))
        qkp = ctx.enter_context(tc.tile_pool(name="qk", bufs=1))
        P = {}  # phase-scoped PSUM pools, resolved at call time

        # ---------------- DMA inputs (parallel queues, ordered by need) ------
        wt = {}
        for nm in ("xh8", "xl8", "wqh", "wql", "wkh", "wkl", "wvh", "wvl"):
            shape = [128, DC, S] if nm.startswith("x") else [128, DC, D]
            wt[nm] = wp.tile(shape, F8, tag=nm, name=nm)
        wo = wp.tile([128, DC, D], F16, tag="wo16")
        # strict demand order; both HWDGE desc-gen (~0.63us each) and the
        # transfer channel serialize globally, so few big DMAs, demand-ordered
        dmas = [
            (wt["xh8"][:, :, 0:SH], t_in["xh8"][:, :, 0:SH]),
            (wt["xl8"][:, :, 0:SH], t_in["xl8"][:, :, 0:SH]),
            (wt["wqh"][:], t_in["wqh"][...]),
            (wt["wql"][:], t_in["wql"][...]),
            (wt["wkh"][:], t_in["wkh"][...]),
            (wt["wkl"][:], t_in["wkl"][...]),
            (wt["xh8"][:, :, SH:S], t_in["xh8"][:, :, SH:S]),
            (wt["xl8"][:, :, SH:S], t_in["xl8"][:, :, SH:S]),
            (wt["wvh"][:], t_in["wvh"][...]),
            (wt["wvl"][:], t_in["wvl"][...]),
            (wo[:], t_in["wo16"][...]),
        ]
        qs = [nc.sync, nc.scalar, nc.gpsimd]
        for i, (dst, src_) in enumerate(dmas):
            qs[i % 3].dma_start(out=dst, in_=src_)
        xh8, xl8 = wt["xh8"], wt["xl8"]

        # ---------------- constants ----------------
        ident = constp.tile([128, 128], F16, tag="ident")
        identf = constp.tile([128, 128], FP, tag="identf")
        make_identity(nc, identf[:])
        nc.gpsimd.tensor_copy(ident[:], identf[:])
        ones16 = constp.tile([128, 1], F16, tag="ones16")
        nc.gpsimd.memset(ones16[:], 1.0)

        QT = qkp.tile([128, DC, SH], F16, tag="QT")
        KT = qkp.tile([128, DC, S], F16, tag="KT")
        Vaug = qkp.tile([128, SC, H * 65], F16, tag="Vaug")
        # denominator columns: value WS (absorbs the x64 V scale)
        nc.gpsimd.memset(
            Vaug[:].rearrange("p s (h e) -> p s h e", h=H)[:, :, :, 64:65], WS
        )

        if with_bias:
            b_sb = {}
            for nm in ("bq", "bk", "bv"):
                t = constp.tile([128, DC], FP, tag=f"b_{nm}", name=f"b_{nm}")
                for c in range(DC):
                    nc.sync.dma_start(
                        out=t[:, c : c + 1], in_=t_in[nm][c * 128 : (c + 1) * 128, None]
                    )
                b_sb[nm] = t
            for nm in ("bq", "bk"):  # match the x64-scaled q/k outputs
                nc.vector.tensor_scalar_mul(b_sb[nm][:], b_sb[nm][:], WS)
            bo_row = constp.tile([1, D], F16, tag="bo_row")
            bo_f = constp.tile([1, D], FP, tag="bo_f")
            nc.sync.dma_start(out=bo_f[:], in_=t_in["bo"][None, :])
            nc.vector.tensor_copy(bo_row[:], bo_f[:])
            ones1 = constp.tile([1, 128], F16, tag="ones1")
            nc.vector.memset(ones1[:], 1.0)

        def warmup(n, pool):
            # keep the PE p-state ramped through known idle windows
            pwu = pool.tile([128, 128], F16, tag="wu")
            for _ in range(n):
                nc.tensor.transpose(pwu[:], ident[:], ident[:])

        # ---------------- QKV projections (fp8 DoubleRow hi/lo) --------------
        def qt_chunk(c):
            pq = P["ppq"].tile([128, SH], FP, tag="pp")
            for n0 in range(0, SH, 256):
                for kp in range(DC // 2):
                    _mm_hilo(
                        nc, pq[:, n0 : n0 + 256],
                        wt["wqh"][:, 2 * kp : 2 * kp + 2, c * 128 : (c + 1) * 128],
                        wt["wql"][:, 2 * kp : 2 * kp + 2, c * 128 : (c + 1) * 128],
                        xh8[:, 2 * kp : 2 * kp + 2, n0 : n0 + 256],
                        xl8[:, 2 * kp : 2 * kp + 2, n0 : n0 + 256],
                        first=(kp == 0), last=(kp == DC // 2 - 1),
                    )
            if with_bias:
                nc.scalar.activation(
                    QT[:, c, :], pq[:], AF.Identity, bias=b_sb["bq"][:, c : c + 1]
                )
            else:
                nc.scalar.copy(QT[:, c, :], pq[:])

        def kt_chunk(c, sg):
            if True:
                pk = P["ppq"].tile([128, 512], FP, tag="pp")
                for n0 in range(0, 512, 256):
                    s0 = sg * 512 + n0
                    for kp in range(DC // 2):
                        _mm_hilo(
                            nc, pk[:, n0 : n0 + 256],
                            wt["wkh"][:, 2 * kp : 2 * kp + 2, c * 128 : (c + 1) * 128],
                            wt["wkl"][:, 2 * kp : 2 * kp + 2, c * 128 : (c + 1) * 128],
                            xh8[:, 2 * kp : 2 * kp + 2, s0 : s0 + 256],
                            xl8[:, 2 * kp : 2 * kp + 2, s0 : s0 + 256],
                            first=(kp == 0), last=(kp == DC // 2 - 1),
                        )
                if with_bias:
                    nc.vector.tensor_scalar(
                        KT[:, c, sg * 512 : sg * 512 + 512], pk[:],
                        b_sb["bk"][:, c : c + 1], None, ALU.add,
                    )
                else:
                    nc.vector.tensor_copy(KT[:, c, sg * 512 : sg * 512 + 512], pk[:])

        def v_chunk(sc):
            pv = P["ppv"].tile([128, D], FP, tag="ppv")
            for n0, nw in ((0, 256), (256, 256), (512, 256)):
                for kp in range(DC // 2):
                    _mm_hilo(
                        nc, pv[:, n0 : n0 + nw],
                        xh8[:, 2 * kp : 2 * kp + 2, sc * 128 : (sc + 1) * 128],
                        xl8[:, 2 * kp : 2 * kp + 2, sc * 128 : (sc + 1) * 128],
                        wt["wvh"][:, 2 * kp : 2 * kp + 2, n0 : n0 + nw],
                        wt["wvl"][:, 2 * kp : 2 * kp + 2, n0 : n0 + nw],
                        first=(kp == 0), last=(kp == DC // 2 - 1),
                        lo_last="rhs",
                    )
            dst = Vaug[:].rearrange("p s (h e) -> p s h e", h=H)[:, sc, :, 0:64]
            src = pv[:].rearrange("p (h e) -> p h e", h=H)
            eng = VEV_SCHED[sc % len(VEV_SCHED)]
            if eng == "a":
                nc.scalar.copy(dst, src)
            elif eng == "d":
                nc.vector.tensor_copy(dst, src)
            else:
                nc.gpsimd.tensor_copy(dst, src)

        # ---------------- attention per head ----------------
        e16p = ctx.enter_context(
            tc.tile_pool(name="e16", bufs=10 if with_bias else 12))
        sctxp = ctx.enter_context(tc.tile_pool(name="sctx", bufs=1))
        smallp = ctx.enter_context(tc.tile_pool(name="small", bufs=3))
        sbctx = [sctxp.tile([128, D], F16, tag=f"sbctx{qc}", name=f"sbctx{qc}")
                 for qc in range(4)]

        def scores_exp(h, e16):
            hb = 64 * (h % 2)
            hc = h // 2
            for kc in range(SC):
                psc = P["ps"].tile([128, SH], FP, tag="ps")
                nc.tensor.matmul(
                    psc[:],
                    KT[hb : hb + 64, hc, kc * 128 : (kc + 1) * 128],
                    QT[hb : hb + 64, hc, :],
                    start=True, stop=True,
                )
                eng = EXP_SCHED[(h * SC + kc) % len(EXP_SCHED)]
                if eng == "a":
                    nc.scalar.activation(e16[:, kc, :], psc[:], AF.Exp,
                                         scale=SCORE_SCALE)
                else:
                    e_i16 = e16[:, kc, :].bitcast(I16)
                    v_eng = nc.vector if eng == "d" else nc.gpsimd
                    v_eng.tensor_scalar(e_i16, psc[:], SCH_A, SCH_B,
                                        ALU.mult, ALU.add)

        def ctx_head(h, e16):
            pctx = P["pc"].tile([128, 4, 65], FP, tag="pc",
                                 padded_shape=[128, 4, 128])
            for qc in range(4):
                for kc in range(SC):
                    nc.tensor.matmul(
                        pctx[:, qc, :],
                        e16[:, kc, qc * 128 : (qc + 1) * 128],
                        Vaug[:, kc, h * 65 : (h + 1) * 65],
                        start=(kc == 0), stop=(kc == SC - 1),
                    )
            recip = smallp.tile([128, 4], FP, tag="recip")
            nc.vector.reciprocal(recip[:], pctx[:, :, 64:65])
            eng = NRM_SCHED[h % len(NRM_SCHED)]
            for qc in range(4):
                dst = sbctx[qc][:, h * 64 : (h + 1) * 64]
                if eng == "a":
                    nc.scalar.activation(dst, pctx[:, qc, 0:64], AF.Copy,
                                         scale=recip[:, qc : qc + 1])
                else:
                    nc.gpsimd.tensor_scalar(dst, pctx[:, qc, 0:64],
                                            recip[:, qc : qc + 1], None, ALU.mult)

        # ---------------- tail (masked rows): mean_k v ----------------
        def tail_prep():
            # sum_k v via ones^T @ Vaug on the PE; two groups (partitions 0/32)
            # keep each PSUM region within one bank
            pw = P["px"].tile([128, 512], FP, tag="pxw")
            for sc in range(SC):
                nc.tensor.matmul(pw[0:1, 0:390], ones16[:, 0:1], Vaug[:, sc, 0:390],
                                 start=(sc == 0), stop=(sc == SC - 1))
            for sc in range(SC):
                nc.tensor.matmul(pw[32:33, 0:390], ones16[:, 0:1], Vaug[:, sc, 390:780],
                                 start=(sc == 0), stop=(sc == SC - 1))
            vrow = smallp.tile([1, H * 65], FP, tag="vrow", name="vrow")
            nc.scalar.copy(vrow[0:1, 0:390], pw[0:1, 0:390])
            nc.scalar.copy(vrow[0:1, 390:780], pw[32:33, 0:390])
            # strided row read (skip denominator columns) -> [1, 768] f16 / S
            vsb = smallp.tile([1, D], F16, tag="vsb", name="vsb")
            nc.scalar.mul(
                vsb[:].rearrange("p (h e) -> p h e", h=H),
                vrow[:].rearrange("p (h e) -> p h e", h=H)[:, :, 0:64],
                1.0 / (S * WS),
            )
            # "transpose" [1, 768] -> [128, 6] via K=1 rank-1 matmuls
            pxt = P["px"].tile([128, DC], FP, tag="pxt")
            for c in range(DC):
                nc.tensor.matmul(
                    pxt[:, c : c + 1], vsb[0:1, c * 128 : (c + 1) * 128],
                    ones16[0:1, 0:1], start=True, stop=True,
                )
            mvt = smallp.tile([128, DC], F16, tag="mvt", name="mvt")
            if with_bias:
                mvf = smallp.tile([128, DC], FP, tag="mvf", name="mvf")
                nc.vector.tensor_tensor(mvf[:], pxt[:], b_sb["bv"][:], ALU.add)
                nc.vector.tensor_copy(mvt[:], mvf[:])
            else:
                nc.vector.tensor_copy(mvt[:], pxt[:])
            return mvt

        def tail_row(mvt):
            # out_tail [1,768] = mvt^T @ Wo; two groups at partitions 0/32
            pt = P["pt2"].tile([128, 512], FP, tag="pt2")
            for k in range(DC):
                nc.tensor.matmul(
                    pt[0:1, 0:512], mvt[:, k : k + 1], wo[:, k, 0:512],
                    start=(k == 0), stop=(not with_bias and k == DC - 1),
                )
            if with_bias:
                nc.tensor.matmul(pt[0:1, 0:512], ones1[0:1, 0:1],
                                 bo_row[0:1, 0:512], start=False, stop=True)
            for k in range(DC):
                nc.tensor.matmul(
                    pt[32:33, 0:256], mvt[:, k : k + 1], wo[:, k, 512:768],
                    start=(k == 0), stop=(not with_bias and k == DC - 1),
                )
            if with_bias:
                nc.tensor.matmul(pt[32:33, 0:256], ones1[0:1, 0:1],
                                 bo_row[0:1, 512:768], start=False, stop=True)
            trow = smallp.tile([1, D], FP, tag="trow", name="trow")
            nc.scalar.copy(trow[0:1, 0:512], pt[0:1, 0:512])
            nc.scalar.copy(trow[0:1, 512:768], pt[32:33, 0:256])
            ttile = smallp.tile([128, D], FP, tag="ttile", name="ttile")
            nc.gpsimd.partition_broadcast(ttile[:], trow[0:1, :])
            engs = [nc.sync, nc.scalar, nc.gpsimd]
            for i, sc in enumerate(range(SH // 128, SC)):
                engs[i % 3].dma_start(
                    out=out[sc * 128 : (sc + 1) * 128, :], in_=ttile[:]
                )

        # ---------------- out projection per q chunk ----------------
        ctxT = qkp.tile([128, DC, SH], F16, tag="ctxT")

        def out_chunk(qc):
            ptT = P["pT"].tile([128, DC * 128], F16, tag="pxT")
            for c in range(DC):
                nc.tensor.transpose(
                    ptT[:, c * 128 : (c + 1) * 128],
                    sbctx[qc][:, c * 128 : (c + 1) * 128],
                    ident[:],
                )
            dst = ctxT[:, :, qc * 128 : (qc + 1) * 128]
            srcv = ptT[:].rearrange("p (c s) -> p c s", c=DC)
            if qc == 1:
                nc.scalar.copy(dst, srcv)
            elif qc == 2:
                nc.gpsimd.tensor_copy(dst, srcv)
            else:
                nc.vector.tensor_copy(dst, srcv)
            po = P["po"].tile([128, D], FP, tag="ppo",
                              padded_shape=[128, 1024])
            for n0, nw in ((0, 512), (512, 256)):
                for k in range(DC):
                    nc.tensor.matmul(
                        po[:, n0 : n0 + nw],
                        ctxT[:, k, qc * 128 : (qc + 1) * 128],
                        wo[:, k, n0 : n0 + nw],
                        start=(k == 0), stop=(not with_bias and k == DC - 1),
                    )
                if with_bias:
                    nc.tensor.matmul(
                        po[:, n0 : n0 + nw], ones1[0:1, 0:128],
                        bo_row[0:1, n0 : n0 + nw], start=False, stop=True,
                    )
            osb = smallp.tile([128, D], FP, tag="osb")
            if qc % 2 == 0:
                nc.scalar.copy(osb[:], po[:])
            else:
                nc.vector.tensor_copy(osb[:], po[:])
            engs = [nc.sync, nc.scalar, nc.gpsimd]
            engs[qc % 3].dma_start(out=out[qc * 128 : (qc + 1) * 128, :], in_=osb[:])

        # ---------------- schedule ----------------
        e16s = {}

        def emit_scores(h):
            e16s[h] = e16p.tile([128, SC, SH], F16, tag="e16", name=f"e16_{h}")
            scores_exp(h, e16s[h])

        with tc.tile_pool(name="ps", bufs=5, space="PSUM") as ps_pool:
            P["ps"] = ps_pool
            with tc.tile_pool(name="ppq", bufs=3, space="PSUM") as ppq_pool:
                P["ppq"] = ppq_pool
                with tc.tile_pool(name="wu1", bufs=1, space="PSUM") as wu_pool:
                    warmup(70, wu_pool)
                mark("qt")
                for c in range(DC):
                    qt_chunk(c)
                mark("kt0")
                for c in range(DC):
                    kt_chunk(c, 0)
                mark("kt1")
                for c in range(DC):
                    kt_chunk(c, 1)
            # V chunks interleaved with early heads' scores+exp; prefetch
            # depth 6 == e16 pool depth
            with tc.tile_pool(name="ppv", bufs=1, space="PSUM") as ppv_pool:
                P["ppv"] = ppv_pool
                mark("v+scores")
                for sc in range(SC):
                    v_chunk(sc)
                    if sc < 6:
                        emit_scores(sc)
            with tc.tile_pool(name="px", bufs=1, space="PSUM") as px_pool:
                P["px"] = px_pool
                mark("tail_prep")
                mvt = tail_prep()
            with tc.tile_pool(name="pc", bufs=3, space="PSUM") as pc_pool:
                P["pc"] = pc_pool
                mark("heads")
                for h in range(H):
                    mark(f"head{h}")
                    ctx_head(h, e16s[h])
                    del e16s[h]
                    if h + 6 < H:
                        emit_scores(h + 6)


        with (
            tc.tile_pool(name="po", bufs=2, space="PSUM") as po_pool,
            tc.tile_pool(name="pT", bufs=2, space="PSUM") as pT_pool,
            tc.tile_pool(name="pt2", bufs=1, space="PSUM") as pt2_pool,
        ):
            P["po"], P["pT"], P["pt2"] = po_pool, pT_pool, pt2_pool
            with tc.tile_pool(name="wu2", bufs=1, space="PSUM") as wu_pool:
                warmup(60, wu_pool)
            mark("tail_row")
            tail_row(mvt)
            mark("out")
            for qc in range(4):
                out_chunk(qc)


def build_nc(with_bias=False):
    nc = bacc.Bacc("TRN2", target_bir_lowering=False, debug=False, num_devices=NCORES)
    t_in = {}
    t_in["xh8"] = nc.dram_tensor("xh8", [128, DC, S], F8, kind="ExternalInput").ap()
    t_in["xl8"] = nc.dram_tensor("xl8", [128, DC, S], F8, kind="ExternalInput").ap()
    for nm in ("wqh", "wql", "wkh", "wkl", "wvh", "wvl"):
        t_in[nm] = nc.dram_tensor(nm, [128, DC, D], F8, kind="ExternalInput").ap()
    t_in["wo16"] = nc.dram_tensor("wo16", [128, DC, D], F16, kind="ExternalInput").ap()
    if with_bias:
        for nm in ("bq", "bk", "bv", "bo"):
            t_in[nm] = nc.dram_tensor(nm, [D], FP, kind="ExternalInput").ap()
    out = nc.dram_tensor("out", [S, D], FP, kind="ExternalOutput").ap()
    with tile.TileContext(nc) as tc:
        _body(tc, out, t_in, with_bias=with_bias)
    nc.compile()
    return nc


def _hilo8(a):
    import ml_dtypes

    hi = a.astype(ml_dtypes.float8_e4m3)
    lo = (a - hi.astype(np.float32)).astype(ml_dtypes.float8_e4m3)
    return hi, lo


def prep_weights(Wq, Wk, Wv, Wo):
    """Host-side: scale, hi/lo split, and [128, DC, D] relayout of weights."""
    def lay(w):  # [D_in, D_out] -> [128, DC, D_out] with partition = d_in % 128
        return np.ascontiguousarray(w.reshape(DC, 128, D).transpose(1, 0, 2))

    def laycc(w):  # [D_in, D_out] -> [DC_out, 128, DC_in, 128] (col-chunk major)
        return np.ascontiguousarray(
            w.reshape(DC, 128, DC, 128).transpose(2, 1, 0, 3))

    outp = {}
    for nm, w in (("wq", Wq), ("wk", Wk), ("wv", Wv)):
        hi, lo = _hilo8(lay(np.asarray(w, np.float32) * WS))
        outp[nm + "h"], outp[nm + "l"] = hi, lo
    outp["wo16"] = lay(np.asarray(Wo, np.float32)).astype(np.float16)
    return outp


def prep_x(x1):
    """Host-side: [S, D] -> fp8 hi/lo xT [128, DC, S]."""
    xT = np.ascontiguousarray(np.asarray(x1, np.float32).T.reshape(DC, 128, S)
                              .transpose(1, 0, 2))
    hi, lo = _hilo8(xT)
    return hi, lo


def kernel(hidden_states, Wq, bq, Wk, bk, Wv, bv, Wo, bo, _trace=False):
    x = np.asarray(hidden_states, np.float32)
    wshared = prep_weights(Wq, Wk, Wv, Wo)
    biases = {nm: np.ascontiguousarray(np.asarray(v, np.float32))
              for nm, v in (("bq", bq), ("bk", bk), ("bv", bv), ("bo", bo))}
    with_bias = any(np.any(v) for v in biases.values())
    nc = build_nc(with_bias=with_bias)
    in_maps = []
    for i in range(NCORES):
        xh, xl = prep_x(x[i])
        m = {"xh8": xh, "xl8": xl, **wshared}
        if with_bias:
            m.update(biases)
        in_maps.append(m)
    res = run_bass_kernel_spmd(nc, in_maps, core_ids=list(range(NCORES)),
                               trace=_trace)
    out = np.stack([res.results[i]["out"] for i in range(NCORES)], axis=0)
    if _trace:
        kernel.last_results = res
    return out


if __name__ == "__main__":
    rng = np.random.default_rng(0)
    ins = {
        "hidden_states": rng.standard_normal((B, S, D), dtype=np.float32),
        **{w: (rng.standard_normal((D, D)) / np.sqrt(D)).astype(np.float32)
           for w in ("Wq", "Wk", "Wv", "Wo")},
        **{b: np.zeros(D, np.float32) for b in ("bq", "bk", "bv", "bo")},
    }
    o = kernel(**ins)
    print("kernel ran, out shape", o.shape)
